# revision 1
# baseline (speedup 1.0000x reference)
"""Self-contained Trainium2 Bass kernel for the DecoConv GNN layer.

kernel(**inputs) takes the full (unsharded) numpy inputs and returns the full
[100000, 64] fp32 output. Internally: shards destination nodes across the 8
NeuronCores, builds + compiles one SPMD Bass/Tile program on first call, and
runs it via concourse's PJRT path on cores 0-7.
"""
import sys
if '/opt/trn_rl_repo' not in sys.path:
    sys.path.insert(0, '/opt/trn_rl_repo')

import numpy as np

# ======================================================================
# environment fixups (walrus single-sync-wait limit, NTFF hook, uploads)
# ======================================================================
"""Split multi-wait instructions in BIR JSON: this container's walrus supports
only ONE sync wait per instruction. Extra waits are moved onto standalone
EventSemaphore instructions inserted immediately before (same engine, in-order)."""
import orjson

# opcodes that must stay glued to the following instruction (weights load + matmul)
_GLUE_PREV = {"TensorLoad", "LoadStationary", "TensorLoadWeights", "LdWeights"}

def split_multiwaits_json(bir_bytes: bytes) -> bytes:
    d = orjson.loads(bir_bytes)
    n_split = 0
    uid = [0]
    for fn in d.get("functions", []):
        for blk in fn.get("blocks", []):
            insts = blk.get("instructions", [])
            out = []
            for inst in insts:
                si = inst.get("sync_info") or {}
                waits = si.get("on_wait") or []
                if len(waits) > 1:
                    n_split += 1
                    pre = []
                    for w in waits:
                        uid[0] += 1
                        pre.append({
                            "debug": inst.get("debug", 0),
                            "engine": inst["engine"],
                            "ins": [],
                            "name": f"{inst['name']}_sw{uid[0]}",
                            "opcode": "EventSemaphore",
                            "outs": [],
                            "sync_info": {"on_update": [], "on_wait": [w]},
                        })
                    si["on_wait"] = []
                    inst["sync_info"] = si
                    # insert before a glued weights-load if present
                    ip = len(out)
                    while ip > 0 and out[ip-1].get("opcode") in _GLUE_PREV and out[ip-1].get("engine") == inst["engine"]:
                        ip -= 1
                    out[ip:ip] = pre
                out.append(inst)
            blk["instructions"] = out
    return orjson.dumps(d), n_split

_installed = False

def _make_ntff_hook(so_path="/opt/axon/libaxon_pjrt.so"):
    import contextlib, ctypes
    lib = ctypes.CDLL(so_path)
    if not hasattr(lib, "axon_start_nrt_profile"):
        return None
    lib.axon_start_nrt_profile.argtypes = [ctypes.POINTER(ctypes.c_int64), ctypes.c_size_t]
    lib.axon_start_nrt_profile.restype = ctypes.c_int64
    lib.axon_stop_nrt_profile.argtypes = [ctypes.c_char_p]
    lib.axon_stop_nrt_profile.restype = ctypes.c_int64

    @contextlib.contextmanager
    def _hook(output_dir, device_ids):
        import jax
        jax.devices()
        if device_ids:
            ids = (ctypes.c_int64 * len(device_ids))(*device_ids)
            rc = lib.axon_start_nrt_profile(ids, len(device_ids))
        else:
            rc = lib.axon_start_nrt_profile(None, 0)
        if rc != 0:
            raise RuntimeError(f"axon_start_nrt_profile rc={rc}")
        try:
            yield
        finally:
            n = lib.axon_stop_nrt_profile(str(output_dir).encode())
            if n < 0:
                raise RuntimeError(f"axon_stop_nrt_profile rc={n}")
    return _hook


def install():
    global _installed
    if _installed:
        return
    from concourse import bass2jax, bass_utils
    orig = bass_utils.compile_bir_kernel
    def patched(ant_bir_str, compile_dir_path, neff_name, **kw):
        fixed, n = split_multiwaits_json(ant_bir_str if isinstance(ant_bir_str, bytes) else ant_bir_str.encode())
        return orig(fixed, compile_dir_path, neff_name=neff_name, **kw)
    bass2jax.compile_bir_kernel = patched

    # antenv.axon_hooks shim so run_bass_kernel_spmd(trace=True) works
    import sys, types
    try:
        import antenv.axon_hooks  # noqa
    except ImportError:
        hook = _make_ntff_hook()
        mod = types.ModuleType("antenv.axon_hooks")
        mod.get_axon_ntff_profile_hook = lambda: hook
        mod.set_axon_ntff_profile_hook = lambda h: None
        sys.modules["antenv.axon_hooks"] = mod
        import antenv
        antenv.axon_hooks = mod

    # no-op the artifact upload (no bucket access in this sandbox)
    bass_utils.upload_artifacts = lambda tmpdir: f"local:{tmpdir}"
    _installed = True


# ======================================================================
# kernel build + host pre/post processing
# ======================================================================
"""GNN message-passing kernel for TRN2 (dest-sharded SpMM + Linear + residual + BN + ReLU).

Layout strategy (per core):
- Destination nodes sharded: core c owns rows [c*S, (c+1)*S).
- Host groups the core's edges by (128-dest tile, source bucket), pads each
  group's edge list to a multiple of 128, and orders tiles by descending edge
  count so all cores share one SPMD program (per-position block counts = max
  across cores). Source buckets of 25,000 rows keep gather indices in int16
  range for dma_gather.
- x is stored in HBM as [N, 128] bf16 rows (features in [:64], zero pad to a
  256-byte row so dma_gather's elem-size constraint holds).
- Device: per (tile, bucket), dma_gather fetches the group's x rows
  (edge k -> partition k%128, block k//128). VectorE builds the one-hot
  scatter matrix M[e, i] = (r_e == i) and y = val * xg ([:, :, :64]); TensorE
  accumulates h1^T = sum_k Y_k^T @ M_k per dest tile in PSUM. Features then
  live on partitions (transposed layout), so Linear (matmul with a bias row),
  residual add, BN stats (free-dim reductions + 128-float AllReduce) and the
  fused scale/shift/ReLU are cheap batched ops.
"""

import numpy as np
import ml_dtypes

BF16 = ml_dtypes.bfloat16
D = 64
TILE = 128
XROW = 128          # padded bf16 row length of x in HBM (256 bytes)
BK = 25000          # source-bucket rows (int16 index range)
GROUP_TILES = 4     # tiles per linear/residual group (512 psum columns)


# ---------------------------------------------------------------- host prep

def host_prep(x, adj_val, adj_row, adj_col, W, b, n_cores):
    N = x.shape[0]
    S = N // n_cores
    assert S * n_cores == N
    n_tiles = (S + TILE - 1) // TILE
    S_pad = n_tiles * TILE
    nbuck = (N + BK - 1) // BK

    adj_row = np.asarray(adj_row)
    adj_col = np.asarray(adj_col)
    adj_val = np.asarray(adj_val)

    core_of_edge = adj_row // S

    edges_by_core = []
    cnt4 = np.zeros((n_cores, n_tiles, nbuck), dtype=np.int64)
    for c in range(n_cores):
        m = core_of_edge == c
        er = adj_row[m] - c * S
        ec = adj_col[m]
        ev = adj_val[m]
        t = er // TILE
        q = ec // BK
        np.add.at(cnt4[c], (t, q), 1)
        edges_by_core.append((er, ec, ev, t, q))

    # sort each core's tiles by total edge count (desc, stable)
    tot = cnt4.sum(2)
    orders = [np.argsort(-tot[c], kind="stable") for c in range(n_cores)]
    nb4 = np.zeros((n_cores, n_tiles, nbuck), dtype=np.int64)
    for c in range(n_cores):
        nb4[c] = (cnt4[c][orders[c]] + TILE - 1) // TILE
    nb4_shared = nb4.max(0)  # [n_tiles(sorted pos), nbuck]
    # every tile needs >= 1 block so its PSUM accumulation group exists
    empty = nb4_shared.sum(1) == 0
    nb4_shared[empty, 0] = 1

    flat = nb4_shared.reshape(-1)
    block_base4 = np.zeros(n_tiles * nbuck, dtype=np.int64)
    block_base4[1:] = np.cumsum(flat)[:-1]
    block_base4 = block_base4.reshape(n_tiles, nbuck)
    B = int(flat.sum())
    tile_blocks = nb4_shared.sum(1)            # total blocks per tile position
    tile_block_base = block_base4[:, 0]        # first block of each tile

    per_core = []
    for c in range(n_cores):
        er, ec, ev, t, q = edges_by_core[c]
        order = orders[c]
        inv_order = np.empty(n_tiles, dtype=np.int64)
        inv_order[order] = np.arange(n_tiles)
        j = inv_order[t]
        gid = j * nbuck + q  # group id in processing order

        val_arr = np.zeros((TILE, B), dtype=BF16)
        r_arr = np.zeros((TILE, B), dtype=BF16)
        idx16 = np.zeros((TILE, B * 8), dtype=np.int16)

        sidx = np.argsort(gid, kind="stable")
        gg = gid[sidx]
        cnt_g = np.bincount(gg, minlength=n_tiles * nbuck)
        start = np.zeros(n_tiles * nbuck, dtype=np.int64)
        start[1:] = np.cumsum(cnt_g)[:-1]
        pos = np.arange(len(gg)) - start[gg]
        blk = block_base4.reshape(-1)[gg] + pos // TILE
        sp = pos % TILE
        val_arr[sp, blk] = ev[sidx].astype(BF16)
        r_arr[sp, blk] = (er[sidx] - t[sidx] * TILE).astype(BF16)
        rel = (ec[sidx] - q[sidx] * BK).astype(np.int16)
        colpos = blk * 8 + (sp % TILE) // 16
        rowpos = sp % 16
        for g in range(8):
            idx16[rowpos + 16 * g, colpos] = rel

        # transposed residual input, tile-order permuted, fp32, + masked ones row
        xres = np.zeros((D, S_pad), dtype=BF16)
        ones_row = np.zeros((1, S_pad), dtype=BF16)
        xc = x[c * S : (c + 1) * S]
        for jpos, tt in enumerate(order):
            lo = tt * TILE
            hi = min(lo + TILE, S)
            nvalid = hi - lo
            xres[:, jpos * TILE : jpos * TILE + nvalid] = xc[lo:hi].T.astype(BF16)
            ones_row[0, jpos * TILE : jpos * TILE + nvalid] = 1.0

        per_core.append(
            dict(val=val_arr, r=r_arr, idx16=idx16, xres=xres, ones=ones_row,
                 order=order)
        )

    waug = np.zeros((D + 1, D), dtype=BF16)
    waug[:D] = np.asarray(W, dtype=np.float32).T
    waug[D] = np.asarray(b, dtype=np.float32)
    x_pad = np.zeros((N, XROW), dtype=BF16)
    x_pad[:, :D] = np.asarray(x).astype(BF16)

    meta = dict(N=N, S=S, n_tiles=n_tiles, S_pad=S_pad, B=B, nbuck=nbuck,
                nb4_shared=nb4_shared.tolist(),
                block_base4=block_base4.tolist(),
                tile_blocks=[int(v) for v in tile_blocks],
                tile_block_base=[int(v) for v in tile_block_base])
    return meta, per_core, waug, x_pad


def host_post(results, metas, n_cores):
    """Assemble full [N, 64] fp32 output from per-core transposed outputs."""
    meta = metas["meta"]
    S, n_tiles = meta["S"], meta["n_tiles"]
    N = meta["N"]
    out = np.empty((N, D), dtype=np.float32)
    for c in range(n_cores):
        dev = results[c]  # [64, S_pad]
        order = metas["per_core"][c]["order"]
        for jpos, tt in enumerate(order):
            lo = tt * TILE
            hi = min(lo + TILE, S)
            nvalid = hi - lo
            out[c * S + lo : c * S + hi] = dev[:, jpos * TILE : jpos * TILE + nvalid].T
    return out


# ---------------------------------------------------------------- device build

def build_nc(meta, n_cores, eps, replica_groups=None):
    from concourse import bass, mybir, tile

    N = meta["N"]
    S_pad = meta["S_pad"]
    n_tiles = meta["n_tiles"]
    B = meta["B"]
    nbuck = meta["nbuck"]
    nb4_shared = meta["nb4_shared"]
    block_base4 = meta["block_base4"]
    tile_blocks = meta["tile_blocks"]
    tile_block_base = meta["tile_block_base"]
    nbt_max = max(tile_blocks)
    f32 = mybir.dt.float32
    bf16 = mybir.dt.bfloat16
    i16 = mybir.dt.int16
    i32 = mybir.dt.int32

    nc = bass.Bass(debug=False, num_swdge_queues=4)
    x_d = nc.declare_dram_parameter("x_pad", [N, XROW], bf16, isOutput=False)
    idx_d = nc.declare_dram_parameter("idx16", [TILE, B * 8], i16, isOutput=False)
    val_d = nc.declare_dram_parameter("val", [TILE, B], bf16, isOutput=False)
    r_d = nc.declare_dram_parameter("r", [TILE, B], bf16, isOutput=False)
    xres_d = nc.declare_dram_parameter("xres", [D, S_pad], bf16, isOutput=False)
    ones_d = nc.declare_dram_parameter("ones", [1, S_pad], bf16, isOutput=False)
    waug_d = nc.declare_dram_parameter("waug", [D + 1, D], bf16, isOutput=False)
    gam_d = nc.declare_dram_parameter("gam", [D, 1], f32, isOutput=False)
    bet_d = nc.declare_dram_parameter("bet", [D, 1], f32, isOutput=False)
    out_d = nc.declare_dram_parameter("outp", [D, S_pad], f32, isOutput=True)

    cc_in = nc.dram_tensor("cc_in", [D, 2], f32)
    cc_out = nc.dram_tensor("cc_out", [D, 2], f32, addr_space="Shared")

    groups = []
    g0 = 0
    while g0 < n_tiles:
        g1 = min(g0 + GROUP_TILES, n_tiles)
        groups.append((g0 * TILE, (g1 - g0) * TILE))
        g0 = g1
    n_groups = len(groups)

    with tile.TileContext(nc) as tc:
        with (
            tc.tile_pool(name="const", bufs=1) as constp,
            tc.tile_pool(name="big", bufs=1) as bigp,
            tc.tile_pool(name="work", bufs=4) as workp,
            tc.tile_pool(name="mwork", bufs=3) as mworkp,
            tc.tile_pool(name="psA", bufs=4, space="PSUM") as psA,
            tc.tile_pool(name="psB", bufs=2, space="PSUM") as psB,
        ):
            val_sb = bigp.tile([TILE, B], bf16)
            r_sb = bigp.tile([TILE, B], bf16)
            xres_sb = bigp.tile([D, S_pad], bf16)
            h1_sb = bigp.tile([D + 1, S_pad], bf16)
            h3_sb = bigp.tile([D, S_pad], f32)
            waug_sb = constp.tile([D + 1, D], bf16)
            gam_sb = constp.tile([D, 1], f32)
            bet_sb = constp.tile([D, 1], f32)
            iota_i = constp.tile([TILE, TILE], i32)
            iota_b = constp.tile([TILE, TILE], bf16)
            stat_s = constp.tile([D, n_groups], f32)
            stat_q = constp.tile([D, n_groups], f32)

            nc.sync.dma_start(val_sb[:], val_d[:])
            nc.sync.dma_start(r_sb[:], r_d[:])
            nc.sync.dma_start(xres_sb[:], xres_d[:])
            nc.sync.dma_start(h1_sb[D : D + 1, :], ones_d[:])
            nc.sync.dma_start(waug_sb[:], waug_d[:])
            nc.sync.dma_start(gam_sb[:], gam_d[:])
            nc.sync.dma_start(bet_sb[:], bet_d[:])

            nc.gpsimd.iota(iota_i[:], pattern=[[1, TILE]], base=0,
                           channel_multiplier=0)
            nc.vector.tensor_copy(iota_b[:], iota_i[:])

            # one register per distinct gather size (to_reg doesn't cache)
            nidx_regs = {}
            for j in range(n_tiles):
                for q in range(nbuck):
                    nb = nb4_shared[j][q]
                    if nb and nb * TILE not in nidx_regs:
                        nidx_regs[nb * TILE] = nc.gpsimd.to_reg(nb * TILE)

            def emit_group(g):
                off, ncols = groups[g]
                ps2 = psB.tile([D, GROUP_TILES * TILE], f32, tag="ps2")
                nc.tensor.matmul(
                    ps2[:, :ncols],
                    lhsT=waug_sb[:],
                    rhs=h1_sb[:, off : off + ncols],
                    start=True, stop=True,
                )
                nc.vector.tensor_tensor(
                    out=h3_sb[:, off : off + ncols],
                    in0=ps2[:, :ncols],
                    in1=xres_sb[:, off : off + ncols],
                    op=bass.mybir.AluOpType.add,
                )

            for j in range(n_tiles):
                nbt = tile_blocks[j]
                base = tile_block_base[j]
                xg = workp.tile([TILE, nbt_max * XROW], bf16, tag="xg")
                yv = workp.tile([TILE, nbt_max * D], bf16, tag="yv")
                mm = mworkp.tile([TILE, nbt_max * TILE], bf16, tag="mm")

                idx_sb = workp.tile([TILE, nbt_max * 8], i16, tag="idx")
                nc.sync.dma_start(idx_sb[:, : nbt * 8],
                                  idx_d[:, base * 8 : (base + nbt) * 8])
                for q in range(nbuck):
                    nb = nb4_shared[j][q]
                    if nb == 0:
                        continue
                    qbase = block_base4[j][q]
                    rel = qbase - base  # block offset within this tile's xg
                    nrow = min(BK, N - q * BK)
                    nc.gpsimd.dma_gather(
                        out_ap=xg[:, rel * XROW : (rel + nb) * XROW].rearrange(
                            "p (b e) -> p b e", e=XROW),
                        in_ap=x_d[q * BK : q * BK + nrow, :],
                        idxs_ap=idx_sb[:, rel * 8 : (rel + nb) * 8],
                        num_idxs=nb * TILE,
                        num_idxs_reg=nidx_regs[nb * TILE],
                        elem_size=XROW,
                        elem_step=XROW,
                        queue_num=(j * nbuck + q) % 4,
                    )
                # M[p, b, i] = (r[p, b] == i)
                nc.vector.tensor_tensor(
                    out=mm[:, : nbt * TILE].rearrange("p (b i) -> p b i", i=TILE),
                    in0=r_sb[:, base : base + nbt].unsqueeze(2).to_broadcast(
                        [TILE, nbt, TILE]
                    ),
                    in1=iota_b[:].unsqueeze(1).to_broadcast([TILE, nbt, TILE]),
                    op=bass.mybir.AluOpType.is_equal,
                )
                # y[p, b, :] = val[p, b] * xg[p, b, :64]
                nc.vector.tensor_tensor(
                    out=yv[:, : nbt * D].rearrange("p (b d) -> p b d", d=D),
                    in0=xg[:].rearrange("p (b e) -> p b e", e=XROW)[:, :nbt, :D],
                    in1=val_sb[:, base : base + nbt].unsqueeze(2).to_broadcast(
                        [TILE, nbt, D]
                    ),
                    op=bass.mybir.AluOpType.mult,
                )
                ps = psA.tile([D, TILE], f32, tag="ps")
                for k in range(nbt):
                    nc.tensor.matmul(
                        ps[:],
                        lhsT=yv[:, k * D : (k + 1) * D],
                        rhs=mm[:, k * TILE : (k + 1) * TILE],
                        start=(k == 0),
                        stop=(k == nbt - 1),
                    )
                nc.scalar.copy(h1_sb[0:D, j * TILE : (j + 1) * TILE], ps[:])

                if (j + 1) % GROUP_TILES == 0:
                    emit_group((j + 1) // GROUP_TILES - 1)
            if n_tiles % GROUP_TILES != 0:
                emit_group(n_groups - 1)

            # BN stats: sum on DVE, sum-of-squares on ACT (Square + accum)
            for g in range(n_groups):
                off, ncols = groups[g]
                sq_scr = workp.tile([D, GROUP_TILES * TILE], f32, tag="sq")
                nc.scalar.activation(
                    sq_scr[:, :ncols],
                    h3_sb[:, off : off + ncols],
                    bass.mybir.ActivationFunctionType.Square,
                    accum_out=stat_q[:, g : g + 1],
                )
                nc.vector.reduce_sum(
                    stat_s[:, g : g + 1],
                    h3_sb[:, off : off + ncols],
                    axis=bass.mybir.AxisListType.X,
                )
            stats2 = constp.tile([D, 2], f32)
            nc.vector.reduce_sum(stats2[:, 0:1], stat_s[:],
                                 axis=bass.mybir.AxisListType.X)
            nc.vector.reduce_sum(stats2[:, 1:2], stat_q[:],
                                 axis=bass.mybir.AxisListType.X)

            statsg = constp.tile([D, 2], f32)
            if replica_groups is not None:
                nc.gpsimd.dma_start(cc_in[:], stats2[:])
                nc.gpsimd.collective_compute(
                    "AllReduce",
                    bass.mybir.AluOpType.add,
                    replica_groups=replica_groups,
                    ins=[cc_in[:]],
                    outs=[cc_out[:]],
                )
                nc.gpsimd.dma_start(statsg[:], cc_out[:])
            else:
                nc.vector.tensor_copy(statsg[:], stats2[:])

            # finalize BN constants: A = gamma / sqrt(var + eps), Bc = beta - mean*A
            eps_sb = constp.tile([D, 1], f32)
            nc.gpsimd.memset(eps_sb[:], float(eps))
            mean = constp.tile([D, 1], f32)
            esq = constp.tile([D, 1], f32)
            var = constp.tile([D, 1], f32)
            sd = constp.tile([D, 1], f32)
            rsd = constp.tile([D, 1], f32)
            A = constp.tile([D, 1], f32)
            Bc = constp.tile([D, 1], f32)
            inv_n = 1.0 / float(N)
            nc.vector.tensor_scalar_mul(mean[:], statsg[:, 0:1], inv_n)
            nc.vector.tensor_scalar_mul(esq[:], statsg[:, 1:2], inv_n)
            nc.vector.tensor_tensor(out=var[:], in0=mean[:], in1=mean[:],
                                    op=bass.mybir.AluOpType.mult)
            nc.vector.tensor_tensor(out=var[:], in0=esq[:], in1=var[:],
                                    op=bass.mybir.AluOpType.subtract)
            nc.scalar.activation(sd[:], var[:],
                                 bass.mybir.ActivationFunctionType.Sqrt,
                                 bias=eps_sb[:, 0:1], scale=1.0)
            nc.vector.reciprocal(rsd[:], sd[:])
            nc.vector.tensor_tensor(out=A[:], in0=rsd[:], in1=gam_sb[:],
                                    op=bass.mybir.AluOpType.mult)
            nc.vector.tensor_tensor(out=Bc[:], in0=mean[:], in1=A[:],
                                    op=bass.mybir.AluOpType.mult)
            nc.vector.tensor_tensor(out=Bc[:], in0=bet_sb[:], in1=Bc[:],
                                    op=bass.mybir.AluOpType.subtract)

            # apply BN + ReLU in place, then store
            nc.scalar.activation(h3_sb[:], h3_sb[:],
                                 bass.mybir.ActivationFunctionType.Relu,
                                 bias=Bc[:, 0:1], scale=A[:, 0:1])
            nc.sync.dma_start(out_d[:], h3_sb[:])

    # Raw Bass (Tile) skips Bacc's library/ISA lowering passes; without them
    # the extended instructions (DMAGatherAnt) have empty .instr bytes and
    # walrus fails with "ISA wrong length", and no LOAD_LIB is emitted.
    import bass_rust as _bass_rust
    from concourse.library_config import all_libraries, standard
    inst_type_to_lib_mask = {}
    for lib in all_libraries:
        for inst_type in lib.instructions:
            inst_type_to_lib_mask[inst_type] = inst_type_to_lib_mask.get(
                inst_type, 0) | (1 << lib.index)
    _bass_rust.insert_library_loads(
        nc, inst_type_to_lib_mask, len(all_libraries), standard.index)
    mybir.codegen_inst_isa_subclasses(nc)
    return nc


def make_in_maps(meta, per_core, waug, x_pad, gamma, beta, n_cores):
    maps = []
    for c in range(n_cores):
        pc = per_core[c]
        maps.append({
            "x_pad": x_pad,
            "idx16": pc["idx16"],
            "val": pc["val"],
            "r": pc["r"],
            "xres": pc["xres"],
            "ones": pc["ones"],
            "waug": waug,
            "gam": np.asarray(gamma, dtype=np.float32).reshape(D, 1),
            "bet": np.asarray(beta, dtype=np.float32).reshape(D, 1),
        })
    return maps


# ======================================================================
# entry point
# ======================================================================
_CACHE = {}

EPS = 1e-5
N_CORES = 8


def kernel(x, adj_val, W, b, gamma, beta, adj_row, adj_col):
    install()
    x = np.asarray(x); adj_val = np.asarray(adj_val)
    W = np.asarray(W); b = np.asarray(b)
    gamma = np.asarray(gamma); beta = np.asarray(beta)
    adj_row = np.asarray(adj_row).astype(np.int64)
    adj_col = np.asarray(adj_col).astype(np.int64)

    meta, per_core, waug, x_pad = host_prep(
        x, adj_val, adj_row, adj_col, W, b, N_CORES)
    in_maps = make_in_maps(meta, per_core, waug, x_pad, gamma, beta, N_CORES)

    key = (meta["B"], tuple(meta["tile_blocks"]))
    if key not in _CACHE:
        nc = build_nc(meta, N_CORES, EPS,
                      replica_groups=[list(range(N_CORES))])
        _CACHE[key] = nc
    nc = _CACHE[key]

    from concourse.bass_utils import run_bass_kernel_spmd
    res = run_bass_kernel_spmd(nc, in_maps, list(range(N_CORES)))
    out = host_post([res.results[c]["outp"] for c in range(N_CORES)],
                    dict(meta=meta, per_core=per_core), N_CORES)
    return out.astype(np.float32)

